# revision 9
# baseline (speedup 1.0000x reference)
"""Trainium2 Bass kernel for nn_Net_75771813036891 (autoregressive spin sampling).

Math: the masked-linear net is autoregressive — output column i depends only on
sample columns < i, and sample column j is frozen after loop iteration j.  So
the column x[:, i] computed at iteration i of the sampling loop IS the final
output column: no final full forward pass and no per-iteration recompute are
needed.  We keep per-layer pre-activation accumulators A_l (contributions of
all past spins' hidden outputs); per spin we only compute that spin's 20
hidden units per layer.  Accumulator scatter-updates are deferred and batched
over 4-spin windows so the matmul contraction dim is 128 (vs 20).

Layout: hidden dim permuted to spin-major and padded to 32 rows per spin
(padded hidden = 2048) so every per-spin slice starts at a partition offset in
{0,32,64,96} (TRN2 engine constraint).  Per-spin psum targets use
tile_position=(0, 32k).  Batch (16384) is data-parallel over 8 cores
(2048/core), split into 8 chunks of 256 columns processed in interleaved
pairs so one chunk's serial spin chain hides under the other's compute.
All matmuls fp32 (sampling decisions x >= u are precision-critical).
"""

import sys

for _p in ("/opt/trn_rl_repo",):
    if _p not in sys.path:
        sys.path.append(_p)

import numpy as np

import concourse.bacc as bacc
import concourse.bass as bass
import concourse.mybir as mybir
import concourse.tile as tile
from concourse.bass_utils import run_bass_kernel_spmd

# ---------------------------------------------------------------- constants
NS = 64          # spins
HID = 20         # hidden channels per spin
H = NS * HID     # 1280 (tight hidden)
SL = 32          # padded slot per spin
H2 = NS * SL     # 2048 (padded hidden)
B = 16384
NCORES = 8
BPC = B // NCORES          # 2048 batch per core
CH = 256                   # chunk width
NCH = BPC // CH            # 8 chunks
WIN = 4                    # spins per window
NW = NS // WIN             # 16 windows
F32 = mybir.dt.float32

MW = [H2 - 128 * (w + 1) for w in range(NW)]   # deferred M per window (w<NW-1)

# per-window wslab layout: [W1WIN(128) | ISL2..5(4*128) | I6(4) | D6(64, w<15)]
W1O = 0
ISLO = 128
I6O = 128 + 512
D6O = I6O + WIN
WSL_COLS = D6O + NS        # 708 (last window simply leaves D6 zero/unused)

# big deferred slab (streamed per 128-col M-tile):
# for w in 0..14: [D2(M_w) | D3 | D4 | D5]
DSL_OFF = []
_c = 0
for _w in range(NW - 1):
    DSL_OFF.append(_c)
    _c += 4 * MW[_w]
DSL_COLS = _c


def _pad_mid(Wt):
    """(1280,1280) tight -> (2048,2048) padded both dims."""
    W4 = Wt.reshape(NS, HID, NS, HID)
    P = np.zeros((NS, SL, NS, SL), np.float32)
    P[:, :HID, :, :HID] = W4
    return P.reshape(H2, H2)


def _pack_weights(W1, W2, W3, W4, W5, W6):
    f = np.float32
    tril0 = np.tril(np.ones((NS, NS), f), 0)
    tril1 = np.tril(np.ones((NS, NS), f), -1)
    M1 = np.tile(tril1, (HID, 1))
    Mh = np.tile(tril0, (HID, HID))
    M6 = np.tile(tril0, (1, HID))

    p = np.zeros(H, dtype=np.int64)       # new = spin*HID+ch <- orig ch*NS+spin
    for s in range(NS):
        for c in range(HID):
            p[s * HID + c] = c * NS + s

    W1p = (M1 * np.asarray(W1, f))[p]                      # (H, NS)
    Wmid = [_pad_mid((Mh * np.asarray(W, f))[p][:, p]) for W in (W2, W3, W4, W5)]
    W6p = (M6 * np.asarray(W6, f))[:, p]                   # (NS, H)

    W1pad = np.zeros((NS, SL, NS), f)                      # [spin_out, ch, spin_in]
    W1pad[:, :HID, :] = W1p.reshape(NS, HID, NS)
    W1Tp = np.ascontiguousarray(W1pad.reshape(H2, NS).T)   # (NS, H2)

    W6p2 = np.zeros((NS, NS, SL), f)
    W6p2[:, :, :HID] = W6p.reshape(NS, NS, HID)
    W6p2 = W6p2.reshape(NS, H2)                            # (NS, H2) padded cols

    wsl = np.zeros((128, NW * WSL_COLS), f)
    dsl = np.zeros((128, DSL_COLS), f)
    for w in range(NW):
        w0 = WIN * w
        r = slice(128 * w, 128 * (w + 1))                  # this window's padded rows
        b = w * WSL_COLS
        # W1WIN[32j, 32k+m] = W1p[20*(w0+k)+m, w0+j]
        blk = W1pad[w0:w0 + WIN, :, w0:w0 + WIN]           # [k, m, j]
        w1w = np.zeros((WIN, SL, WIN, SL), f)
        w1w[:, 0, :, :] = blk.transpose(2, 0, 1)           # [j, k, m]
        wsl[:, b + W1O: b + W1O + 128] = w1w.reshape(128, 128)
        for li, Wl in enumerate(Wmid):
            wsl[:, b + ISLO + 128 * li: b + ISLO + 128 * (li + 1)] = Wl[r, r].T
        wsl[:, b + I6O: b + I6O + WIN] = W6p2[w0:w0 + WIN, r].T
        if w < NW - 1:
            wsl[:, b + D6O: b + D6O + NS] = W6p2[:, r].T
            do, M = DSL_OFF[w], MW[w]
            for li, Wl in enumerate(Wmid):
                dsl[:, do + li * M: do + (li + 1) * M] = Wl[128 * (w + 1):, r].T
    return W1Tp, wsl, dsl


# ------------------------------------------------------------- device build
def _patched_tile_context(nc):
    # Bacc.compile()'s generate_event_semaphores pass splits multi-wait
    # instructions to the TRN2 1-wait-per-instruction limit, so the plain
    # TileContext is fine as long as the module is a Bacc and compile() runs.
    return tile.TileContext(nc)


def _strided4(t):
    """(128, CH) tile viewed as 4 rows at partition stride 32."""
    return t.rearrange("(a b) c -> a b c", b=SL)[:, 0, :]


def build_nc():
    SIG = mybir.ActivationFunctionType.Sigmoid
    ADD = mybir.AluOpType.add
    GE = mybir.AluOpType.is_ge
    MUL = mybir.AluOpType.mult
    SUB = mybir.AluOpType.subtract

    nc = bacc.Bacc()
    u_d = nc.declare_dram_parameter("U", [NS, BPC], F32, isOutput=False)
    w1_d = nc.declare_dram_parameter("W1TP", [NS, H2], F32, isOutput=False)
    wsl_d = nc.declare_dram_parameter("WSL", [128, NW * WSL_COLS], F32, isOutput=False)
    dsl_d = nc.declare_dram_parameter("DSL", [128, DSL_COLS], F32, isOutput=False)
    x_d = nc.declare_dram_parameter("X", [NS, BPC], F32, isOutput=True)

    from contextlib import ExitStack

    with _patched_tile_context(nc) as tc, ExitStack() as ctx:
        cpool = ctx.enter_context(tc.tile_pool(name="const", bufs=1))
        apool = ctx.enter_context(tc.tile_pool(name="acc", bufs=1))
        wpool = ctx.enter_context(tc.tile_pool(name="wsl", bufs=2))
        dwpool = ctx.enter_context(tc.tile_pool(name="dstream", bufs=6))
        hpool = ctx.enter_context(tc.tile_pool(name="hwin", bufs=2))
        spool = ctx.enter_context(tc.tile_pool(name="stage", bufs=2))
        zpool = ctx.enter_context(tc.tile_pool(name="zpsum", bufs=6, space="PSUM"))
        dpool = ctx.enter_context(tc.tile_pool(name="dpsum", bufs=2, space="PSUM"))

        w1t = cpool.tile([NS, H2], F32, name="w1t", tag="w1t")
        s_t = cpool.tile([NS, BPC], F32, name="s_t", tag="s")
        nc.sync.dma_start(out=w1t, in_=w1_d[:])
        nc.vector.memset(s_t, 0.0)

        # A[c][li][w] valid for w in 1..NW-1 (window w reads its own tile)
        A = [
            [
                [None] + [
                    apool.tile([128, CH], F32, name=f"A{c}_{li}_{w}",
                               tag=f"A{c}_{li}_{w}")
                    for w in range(1, NW)
                ]
                for li in range(4)
            ]
            for c in range(2)
        ]
        A6 = [apool.tile([NS, CH], F32, name=f"A6_{c}", tag=f"A6_{c}")
              for c in range(2)]
        s_stage = [cpool.tile([128, CH], F32, name=f"sstg{c}", tag=f"sstg{c}")
                   for c in range(2)]
        nc.vector.memset(s_stage[0], 0.0)
        nc.vector.memset(s_stage[1], 0.0)

        for pair in range(NCH // 2):
            css = [slice((2 * pair + c) * CH, (2 * pair + c + 1) * CH)
                   for c in range(2)]
            for w in range(NW):
                w0 = WIN * w
                b = 0  # within wsl tile
                wsl = wpool.tile([128, WSL_COLS], F32, name="wsl", tag="wsl")
                nc.sync.dma_start(
                    out=wsl,
                    in_=wsl_d[:, w * WSL_COLS:(w + 1) * WSL_COLS],
                )
                hb = [
                    [hpool.tile([128, CH], F32, name=f"h{c}_{l}", tag=f"h{c}_{l}")
                     for l in range(5)]
                    for c in range(2)
                ]
                ustg, xstg, a6stg = [], [], []
                for c in range(2):
                    ut = spool.tile([128, CH], F32, name=f"u{c}", tag=f"u{c}")
                    nc.sync.dma_start(out=_strided4(ut), in_=u_d[w0:w0 + WIN, css[c]])
                    ustg.append(ut)
                    xstg.append(spool.tile([128, CH], F32, name=f"x{c}", tag=f"x{c}"))
                    if w > 0:
                        at = spool.tile([128, CH], F32, name=f"a6s{c}", tag=f"a6s{c}")
                        nc.sync.dma_start(out=_strided4(at), in_=A6[c][w0:w0 + WIN, :])
                        a6stg.append(at)
                    else:
                        a6stg.append(None)
                for k in range(WIN):
                    i = w0 + k
                    q = SL * k
                    for c in range(2):
                        cs = css[c]
                        # ---- fc1 (history via s_t + window part via s_stage)
                        z = zpool.tile([128, CH], F32, name="z", tag="z")
                        nc.tensor.matmul(
                            z[q:q + SL, :],
                            w1t[:, 128 * w + q: 128 * w + q + SL],
                            s_t[:, cs],
                            start=True, stop=(k == 0), tile_position=(0, q),
                        )
                        if k > 0:
                            nc.tensor.matmul(
                                z[q:q + SL, :],
                                wsl[0:q, W1O + q: W1O + q + SL],
                                s_stage[c][0:q, :],
                                start=False, stop=True, tile_position=(0, q),
                            )
                        nc.scalar.activation(hb[c][0][q:q + SL, :], z[q:q + SL, :], SIG)
                        # ---- fc2..fc5
                        for li in range(4):
                            z = zpool.tile([128, CH], F32, name="z", tag="z")
                            nc.tensor.matmul(
                                z[q:q + SL, :],
                                wsl[0:q + SL, ISLO + 128 * li + q: ISLO + 128 * li + q + SL],
                                hb[c][li][0:q + SL, :],
                                start=True, stop=True, tile_position=(0, q),
                            )
                            if w > 0:
                                nc.vector.tensor_tensor(
                                    z[q:q + SL, :], z[q:q + SL, :],
                                    A[c][li][w][q:q + SL, :], ADD,
                                )
                            nc.scalar.activation(
                                hb[c][li + 1][q:q + SL, :], z[q:q + SL, :], SIG
                            )
                        # ---- fc6 + sample update
                        z = zpool.tile([128, CH], F32, name="z", tag="z")
                        nc.tensor.matmul(
                            z[q:q + 1, :],
                            wsl[0:q + SL, I6O + k: I6O + k + 1],
                            hb[c][4][0:q + SL, :],
                            start=True, stop=True, tile_position=(0, q),
                        )
                        if w > 0:
                            nc.vector.tensor_tensor(
                                z[q:q + 1, :], z[q:q + 1, :],
                                a6stg[c][q:q + 1, :], ADD,
                            )
                        nc.scalar.activation(xstg[c][q:q + 1, :], z[q:q + 1, :], SIG)
                        nc.vector.tensor_tensor(
                            s_stage[c][q:q + 1, :], xstg[c][q:q + 1, :],
                            ustg[c][q:q + 1, :], GE,
                        )
                        nc.vector.tensor_scalar(
                            s_stage[c][q:q + 1, :], s_stage[c][q:q + 1, :],
                            2.0, 1.0, MUL, SUB,
                        )
                # ---- window end: flush x and s, run deferred scatter
                for c in range(2):
                    nc.sync.dma_start(out=x_d[w0:w0 + WIN, css[c]], in_=_strided4(xstg[c]))
                    nc.sync.dma_start(out=s_t[w0:w0 + WIN, css[c]], in_=_strided4(s_stage[c]))
                if w < NW - 1:
                    do, M = DSL_OFF[w], MW[w]
                    for li in range(4):
                        for t in range(M // 128):
                            dt = dwpool.tile([128, 128], F32, name="dt", tag="dt")
                            nc.sync.dma_start(
                                out=dt,
                                in_=dsl_d[:, do + li * M + 128 * t: do + li * M + 128 * (t + 1)],
                            )
                            for c in range(2):
                                dp = dpool.tile([128, CH], F32, name="dp", tag="dp")
                                nc.tensor.matmul(
                                    dp, dt, hb[c][li], start=True, stop=True
                                )
                                dst = A[c][li][w + 1 + t]
                                if w == 0:
                                    nc.vector.tensor_copy(dst, dp)
                                else:
                                    nc.vector.tensor_tensor(dst, dst, dp, ADD)
                    for c in range(2):
                        dp = dpool.tile([128, CH], F32, name="dp", tag="dp")
                        nc.tensor.matmul(
                            dp[:NS, :], wsl[:, D6O: D6O + NS], hb[c][4],
                            start=True, stop=True,
                        )
                        if w == 0:
                            nc.vector.tensor_copy(A6[c], dp[:NS, :])
                        else:
                            nc.vector.tensor_tensor(A6[c], A6[c], dp[:NS, :], ADD)
    nc.compile()
    return nc


_NC = None


def _get_nc():
    global _NC
    if _NC is None:
        _NC = build_nc()
    return _NC


def run(inputs, trace=False):
    u = np.asarray(inputs["u"], np.float32)
    W1Tp, wsl, dsl = _pack_weights(
        inputs["W1"], inputs["W2"], inputs["W3"],
        inputs["W4"], inputs["W5"], inputs["W6"],
    )
    in_maps = [
        {
            "U": np.ascontiguousarray(u[:, c * BPC:(c + 1) * BPC]),
            "W1TP": W1Tp,
            "WSL": wsl,
            "DSL": dsl,
        }
        for c in range(NCORES)
    ]
    nc = _get_nc()
    res = run_bass_kernel_spmd(nc, in_maps, list(range(NCORES)), trace=trace)
    out = np.concatenate(
        [res.results[c]["X"].T for c in range(NCORES)], axis=0
    ).astype(np.float32)
    return out, res


def kernel(**inputs) -> np.ndarray:
    out, _ = run(inputs, trace=False)
    return out
